# revision 11
# baseline (speedup 1.0000x reference)
"""CrossAttentionFusion Bass kernel for 8 TRN2 NeuronCores.

Reference computation (T=4096, B=64, D=64):
    q = eeg @ Wq.T + bq ; k = fnirs @ Wk.T + bk ; v = fnirs @ Wv.T + bv
    score = sum(q*k, -1) * D**-0.5        # [T, B, 1]
    attn = softmax(score, axis=0)         # over T
    out = eeg + attn * v

Strategy:
  - Data-parallel over batch: core c handles batches [8c, 8c+8).
  - Algebraic fold: score*SCALE = x^T G y + w.x + u.y (+const, dropped —
    softmax shift-invariant), with G = SCALE*Wq^T@Wk, w = SCALE*Wq^T@bk,
    u = SCALE*Wk^T@bq.  This removes the q/k projections entirely.
  - Everything on-device runs in [feature, token] (transposed) layout; the
    host packs eeg/fnirs into stacked [128, 512] tiles (x on partitions
    0:64, y on 64:128) so ONE block-diagonal matmul [[G,0],[0,Wv^T]]
    produces z (=G^T x) and v for 512 tokens at once.
  - Per-batch masked reduce-matmuls (column-one-hot lhsT) accumulate
    scores for all 8 local batches into a single PSUM tile [8, 512].
  - Softmax on [8, 4096] (batch on partitions): one DVE max, one ACT
    Exp with per-partition bias and fused row-sum, reciprocal, scale.
  - Pass B: PE ones-matmul broadcasts attn rows across 64 partitions,
    DVE multiplies by v, GpSimd adds the eeg residual, DMA out.
  - eeg (transposed) and v are kept resident in SBUF between passes, so
    HBM traffic is the minimum 2 reads + 1 write of [T,8,64] per core.
"""

import sys

sys.path.insert(0, "/opt/trn_rl_repo")

import numpy as np

import concourse.bass as bass
import concourse.tile as tile
from concourse import bacc, mybir

T, B, D = 4096, 64, 64
N_CORES = 8
BC = B // N_CORES  # 8 batches per core
NCH = 8  # chunks along T
CH = T // NCH  # 512 tokens per chunk
NPAIR = NCH // 2  # chunk pairs (for 128-partition packing)
SCALE = float(D) ** -0.5
F32 = mybir.dt.float32

_CACHE = {}


def _build_nc():
    nc = bacc.Bacc(
        "TRN2", target_bir_lowering=False, debug=False, num_devices=N_CORES
    )

    xy_d = nc.dram_tensor("XY", [NCH, BC, 128, CH], F32, kind="ExternalInput").ap()
    big_d = nc.dram_tensor("BIG", [128, 128], F32, kind="ExternalInput").ap()
    mask1_d = nc.dram_tensor("MASK1", [64, 64], F32, kind="ExternalInput").ap()
    maskw_d = nc.dram_tensor("MASKW", [64, 64], F32, kind="ExternalInput").ap()
    u_d = nc.dram_tensor("UVEC", [128, 1], F32, kind="ExternalInput").ap()
    bv_d = nc.dram_tensor("BV2", [128, 1], F32, kind="ExternalInput").ap()
    bcast_d = nc.dram_tensor("BCAST", [BC, BC * 64], F32, kind="ExternalInput").ap()
    out_d = nc.dram_tensor(
        "OUT", [BC, NPAIR, 128, CH], F32, kind="ExternalOutput"
    ).ap()

    with tile.TileContext(nc) as tc:
        with (
            tc.tile_pool(name="consts", bufs=1) as consts,
            tc.tile_pool(name="store", bufs=1) as store,
            tc.tile_pool(name="xy", bufs=4) as xyp,
            tc.tile_pool(name="m", bufs=3) as mp,
            tc.tile_pool(name="sm", bufs=1) as smp,
            tc.tile_pool(name="passb", bufs=3) as pbp,
            tc.tile_pool(name="pzv", bufs=2, space="PSUM") as pzvp,
            tc.tile_pool(name="psc", bufs=2, space="PSUM") as pscp,
            tc.tile_pool(name="pa2", bufs=2, space="PSUM") as pa2p,
        ):
            big_s = consts.tile([128, 128], F32)
            nc.sync.dma_start(big_s[:], big_d[:])
            mask1_s = consts.tile([64, 64], F32)
            nc.sync.dma_start(mask1_s[:], mask1_d[:])
            maskw_s = consts.tile([64, 64], F32)
            nc.sync.dma_start(maskw_s[:], maskw_d[:])
            u_s = consts.tile([128, 1], F32)
            nc.sync.dma_start(u_s[:], u_d[:])
            bv_s = consts.tile([128, 1], F32)
            nc.sync.dma_start(bv_s[:], bv_d[:])
            bcast_s = consts.tile([BC, BC * 64], F32)
            nc.sync.dma_start(bcast_s[:], bcast_d[:])

            # persistent stores: eeg^T and v, packed as 32 x [128, CH] each
            xstore = store.tile([128, NPAIR * BC * CH], F32)  # 64 KB/part
            vstore = store.tile([128, NPAIR * BC * CH], F32)  # 64 KB/part
            scores_s = smp.tile([BC, T], F32)

            def stsl(bi, pi):
                # column slice of x/v store for (batch bi, chunk-pair pi)
                return slice((bi * NPAIR + pi) * CH, (bi * NPAIR + pi + 1) * CH)

            # ---------------- pass A ----------------
            for c in range(NCH):
                psc = pscp.tile([BC, CH], F32, tag="psc")
                for b in range(BC):
                    xy = xyp.tile([128, CH], F32, tag="xy")
                    nc.sync.dma_start(xy[:], xy_d[c, b])
                    pzv = pzvp.tile([128, CH], F32, tag="pzv")
                    nc.tensor.matmul(
                        pzv[:], big_s[:], xy[:], start=True, stop=True
                    )
                    half = slice((c % 2) * 64, (c % 2) * 64 + 64)
                    # keep eeg^T for the pass-B residual
                    nc.gpsimd.tensor_copy(
                        xstore[half, stsl(b, c // 2)], xy[0:64, :]
                    )
                    # v = Wv@y + bv, keep for pass B
                    nc.scalar.activation(
                        vstore[half, stsl(b, c // 2)],
                        pzv[64:128, :],
                        mybir.ActivationFunctionType.Identity,
                        bias=bv_s[half, :],
                    )
                    # m = (z + u) * y
                    m = mp.tile([64, CH], F32, tag="m")
                    nc.vector.scalar_tensor_tensor(
                        m[:],
                        pzv[0:64, :],
                        u_s[64:128, :],
                        xy[64:128, :],
                        op0=mybir.AluOpType.add,
                        op1=mybir.AluOpType.mult,
                    )
                    # scores[b, :] += sum_d m  (ones one-hot col b)
                    nc.tensor.matmul(
                        psc[:],
                        mask1_s[:, b * 8 : b * 8 + 8],
                        m[:],
                        start=(b == 0),
                        stop=False,
                    )
                    # scores[b, :] += w.x
                    nc.tensor.matmul(
                        psc[:],
                        maskw_s[:, b * 8 : b * 8 + 8],
                        xy[0:64, :],
                        start=False,
                        stop=(b == BC - 1),
                    )
                nc.scalar.activation(
                    scores_s[:, c * CH : (c + 1) * CH],
                    psc[:],
                    mybir.ActivationFunctionType.Identity,
                    bias=0.0,
                )

            # ---------------- softmax over T (free axis) ----------------
            mx = smp.tile([BC, 1], F32)
            nc.vector.tensor_reduce(
                mx[:], scores_s[:], axis=mybir.AxisListType.X, op=mybir.AluOpType.max
            )
            nmx = smp.tile([BC, 1], F32)
            nc.vector.tensor_scalar_mul(nmx[:], mx[:], -1.0)
            attn = smp.tile([BC, T], F32)
            zsum = smp.tile([BC, 1], F32)
            nc.scalar.activation(
                attn[:],
                scores_s[:],
                mybir.ActivationFunctionType.Exp,
                bias=nmx[:],
                accum_out=zsum[:],
            )
            rz = smp.tile([BC, 1], F32)
            nc.vector.reciprocal(rz[:], zsum[:])
            nc.vector.tensor_scalar_mul(attn[:], attn[:], rz[:])

            # ---------------- pass B ----------------
            for b in range(BC):
                for p in range(NPAIR):
                    pa2 = pa2p.tile([128, CH], F32, tag="pa2")
                    for h in range(2):
                        cs = (2 * p + h) * CH
                        nc.tensor.matmul(
                            pa2[h * 64 : h * 64 + 64, :],
                            bcast_s[:, b * 64 : (b + 1) * 64],
                            attn[:, cs : cs + CH],
                            start=True,
                            stop=True,
                        )
                    tav = pbp.tile([128, CH], F32, tag="tav")
                    nc.vector.tensor_mul(tav[:], pa2[:], vstore[:, stsl(b, p)])
                    o2 = pbp.tile([128, CH], F32, tag="o2")
                    nc.gpsimd.tensor_add(o2[:], tav[:], xstore[:, stsl(b, p)])
                    nc.sync.dma_start(out_d[b, p], o2[:])

    nc.compile()
    return nc


def _get_nc():
    if "nc" not in _CACHE:
        _CACHE["nc"] = _build_nc()
    return _CACHE["nc"]


def _host_constants(Wq, bq, Wk, bk, Wv, bv):
    Wq64, Wk64, Wv64 = (np.asarray(a, np.float64) for a in (Wq, Wk, Wv))
    bq64, bk64 = np.asarray(bq, np.float64), np.asarray(bk, np.float64)
    G = (SCALE * (Wq64.T @ Wk64)).astype(np.float32)  # [d, e]
    w = (SCALE * (Wq64.T @ bk64)).astype(np.float32)  # [64]
    u = (SCALE * (Wk64.T @ bq64)).astype(np.float32)  # [64]

    BIG = np.zeros((128, 128), np.float32)
    BIG[0:64, 0:64] = G  # z[e,n] = sum_d G[d,e] x[d,n]
    BIG[64:128, 64:128] = np.asarray(Wv, np.float32).T  # v[o,n] = sum_e Wv[o,e] y[e,n]

    MASK1 = np.zeros((64, 64), np.float32)
    MASKW = np.zeros((64, 64), np.float32)
    for b in range(BC):
        MASK1[:, b * 8 + b] = 1.0
        MASKW[:, b * 8 + b] = w
    U = np.tile(u.reshape(64, 1), (2, 1)).astype(np.float32)
    BV2 = np.tile(np.asarray(bv, np.float32).reshape(64, 1), (2, 1))
    BCAST = np.zeros((BC, BC * 64), np.float32)
    for b in range(BC):
        BCAST[b, b * 64 : (b + 1) * 64] = 1.0
    return BIG, MASK1, MASKW, U, BV2, BCAST


def _pack_inputs(eeg, fnirs):
    # [T, B, D] -> [core, chunk, b, feat, tok]
    def tr(x):
        x = np.asarray(x, np.float32).reshape(NCH, CH, N_CORES, BC, D)
        return np.ascontiguousarray(x.transpose(2, 0, 3, 4, 1))

    e, f = tr(eeg), tr(fnirs)
    XY = np.empty((N_CORES, NCH, BC, 128, CH), np.float32)
    XY[:, :, :, 0:64, :] = e
    XY[:, :, :, 64:128, :] = f
    return XY


def _unpack_output(outs):
    # outs: list of [BC, NPAIR, 128, CH] per core -> [T, B, D]
    o = np.stack(outs)  # [core, b, pair, 128, tok]
    o = o.reshape(N_CORES, BC, NPAIR, 2, D, CH)  # [core, b, pair, half, d, tok]
    o = o.transpose(2, 3, 5, 0, 1, 4)  # [pair, half, tok, core, b, d]
    return np.ascontiguousarray(o.reshape(T, B, D))


def _run(eeg, fnirs, Wq, bq, Wk, bk, Wv, bv, **spmd_kwargs):
    from concourse.bass_utils import run_bass_kernel_spmd

    nc = _get_nc()
    BIG, MASK1, MASKW, U, BV2, BCAST = _host_constants(Wq, bq, Wk, bk, Wv, bv)
    XY = _pack_inputs(eeg, fnirs)
    in_maps = [
        {
            "XY": np.ascontiguousarray(XY[c]),
            "BIG": BIG,
            "MASK1": MASK1,
            "MASKW": MASKW,
            "UVEC": U,
            "BV2": BV2,
            "BCAST": BCAST,
        }
        for c in range(N_CORES)
    ]
    res = run_bass_kernel_spmd(nc, in_maps, list(range(N_CORES)), **spmd_kwargs)
    return _unpack_output([res.results[c]["OUT"] for c in range(N_CORES)]), res


def kernel(eeg, fnirs, Wq, bq, Wk, bk, Wv, bv):
    return _run(eeg, fnirs, Wq, bq, Wk, bk, Wv, bv)[0]


# revision 12
# speedup vs baseline: 1.0791x; 1.0791x over previous
"""CrossAttentionFusion Bass kernel for 8 TRN2 NeuronCores.

Reference computation (T=4096, B=64, D=64):
    q = eeg @ Wq.T + bq ; k = fnirs @ Wk.T + bk ; v = fnirs @ Wv.T + bv
    score = sum(q*k, -1) * D**-0.5        # [T, B, 1]
    attn = softmax(score, axis=0)         # over T
    out = eeg + attn * v

Strategy:
  - Data-parallel over batch: core c handles batches [8c, 8c+8).
  - Algebraic fold: score*SCALE = x^T G y + w.x + u.y (+const, dropped —
    softmax shift-invariant), with G = SCALE*Wq^T@Wk, w = SCALE*Wq^T@bk,
    u = SCALE*Wk^T@bq.  This removes the q/k projections entirely.
  - Everything on-device runs in [feature, token] (transposed) layout; the
    host packs eeg/fnirs into stacked bf16 [128, 512] tiles (x on
    partitions 0:64, y on 64:128) so ONE block-diagonal bf16 matmul
    [[G,0],[0,Wv^T]] (fp32 PSUM accumulate) produces z (=G^T x) and v for
    512 tokens at once.
  - Per-batch masked reduce-matmuls (column-one-hot lhsT) accumulate
    scores for all 8 local batches into a single PSUM tile [8, 512].
  - Softmax on [8, 4096] (batch on partitions): one DVE max, one ACT
    Exp with per-partition bias and fused row-sum, reciprocal, scale.
  - Pass B: PE one-hot-row matmul broadcasts attn rows across 64
    partitions, DVE multiplies by v (fp32), a gpsimd accumulate-DMA adds
    the fp32 eeg residual (prefetched to SBUF during pass A), DMA out.
"""

import sys

sys.path.insert(0, "/opt/trn_rl_repo")

import ml_dtypes
import numpy as np

import concourse.bass as bass
import concourse.tile as tile
from concourse import bacc, mybir

T, B, D = 4096, 64, 64
N_CORES = 8
BC = B // N_CORES  # 8 batches per core
NCH = 8  # chunks along T
CH = T // NCH  # 512 tokens per chunk
NPAIR = NCH // 2  # chunk pairs (for 128-partition packing)
SCALE = float(D) ** -0.5
F32 = mybir.dt.float32
BF16 = mybir.dt.bfloat16
NPBF16 = ml_dtypes.bfloat16

_CACHE = {}


def _build_nc():
    nc = bacc.Bacc(
        "TRN2", target_bir_lowering=False, debug=False, num_devices=N_CORES
    )

    xy_d = nc.dram_tensor("XY", [NCH, BC, 128, CH], BF16, kind="ExternalInput").ap()
    eegr_d = nc.dram_tensor(
        "EEGR", [BC, NPAIR, 128, CH], F32, kind="ExternalInput"
    ).ap()
    big_d = nc.dram_tensor("BIG", [128, 128], BF16, kind="ExternalInput").ap()
    mask1_d = nc.dram_tensor("MASK1", [64, 64], BF16, kind="ExternalInput").ap()
    maskw_d = nc.dram_tensor("MASKW", [64, 64], BF16, kind="ExternalInput").ap()
    u_d = nc.dram_tensor("UVEC", [128, 1], F32, kind="ExternalInput").ap()
    bv_d = nc.dram_tensor("BV2", [128, 1], F32, kind="ExternalInput").ap()
    bcast_d = nc.dram_tensor(
        "BCAST", [BC, BC * 64], BF16, kind="ExternalInput"
    ).ap()
    out_d = nc.dram_tensor(
        "OUT", [BC, NPAIR, 128, CH], F32, kind="ExternalOutput"
    ).ap()

    with tile.TileContext(nc) as tc:
        with (
            tc.tile_pool(name="consts", bufs=1) as consts,
            tc.tile_pool(name="store", bufs=1) as store,
            tc.tile_pool(name="xy", bufs=4) as xyp,
            tc.tile_pool(name="m", bufs=3) as mp,
            tc.tile_pool(name="sm", bufs=1) as smp,
            tc.tile_pool(name="passb", bufs=3) as pbp,
            tc.tile_pool(name="pzv", bufs=2, space="PSUM") as pzvp,
            tc.tile_pool(name="psc", bufs=2, space="PSUM") as pscp,
            tc.tile_pool(name="pa2", bufs=2, space="PSUM") as pa2p,
        ):
            big_s = consts.tile([128, 128], BF16)
            nc.sync.dma_start(big_s[:], big_d[:])
            mask1_s = consts.tile([64, 64], BF16)
            nc.sync.dma_start(mask1_s[:], mask1_d[:])
            maskw_s = consts.tile([64, 64], BF16)
            nc.sync.dma_start(maskw_s[:], maskw_d[:])
            u_s = consts.tile([128, 1], F32)
            nc.sync.dma_start(u_s[:], u_d[:])
            bv_s = consts.tile([128, 1], F32)
            nc.sync.dma_start(bv_s[:], bv_d[:])
            bcast_s = consts.tile([BC, BC * 64], BF16)
            nc.sync.dma_start(bcast_s[:], bcast_d[:])

            # persistent stores: eeg (fp32 residual) and v, 32 x [128, CH] each
            xstore = store.tile([128, NPAIR * BC * CH], F32)  # 64 KB/part
            vstore = store.tile([128, NPAIR * BC * CH], F32)  # 64 KB/part
            scores_s = smp.tile([BC, T], F32)

            def stsl(bi, pi):
                return slice((bi * NPAIR + pi) * CH, (bi * NPAIR + pi + 1) * CH)

            # prefetch fp32 eeg residual for pass B (overlaps pass A compute)
            for b in range(BC):
                for p in range(NPAIR):
                    nc.sync.dma_start(xstore[:, stsl(b, p)], eegr_d[b, p])

            # ---------------- pass A ----------------
            for c in range(NCH):
                psc = pscp.tile([BC, CH], F32, tag="psc")
                for b in range(BC):
                    xy = xyp.tile([128, CH], BF16, tag="xy")
                    nc.sync.dma_start(xy[:], xy_d[c, b])
                    pzv = pzvp.tile([128, CH], F32, tag="pzv")
                    nc.tensor.matmul(
                        pzv[:], big_s[:], xy[:], start=True, stop=True
                    )
                    half = slice((c % 2) * 64, (c % 2) * 64 + 64)
                    # v = Wv@y + bv, keep for pass B
                    nc.scalar.activation(
                        vstore[half, stsl(b, c // 2)],
                        pzv[64:128, :],
                        mybir.ActivationFunctionType.Identity,
                        bias=bv_s[half, :],
                    )
                    # m = (z + u) * y
                    m = mp.tile([64, CH], BF16, tag="m")
                    nc.vector.scalar_tensor_tensor(
                        m[:],
                        pzv[0:64, :],
                        u_s[64:128, :],
                        xy[64:128, :],
                        op0=mybir.AluOpType.add,
                        op1=mybir.AluOpType.mult,
                    )
                    # scores[b, :] += sum_d m  (ones one-hot col b)
                    nc.tensor.matmul(
                        psc[:],
                        mask1_s[:, b * 8 : b * 8 + 8],
                        m[:],
                        start=(b == 0),
                        stop=False,
                    )
                    # scores[b, :] += w.x
                    nc.tensor.matmul(
                        psc[:],
                        maskw_s[:, b * 8 : b * 8 + 8],
                        xy[0:64, :],
                        start=False,
                        stop=(b == BC - 1),
                    )
                nc.scalar.activation(
                    scores_s[:, c * CH : (c + 1) * CH],
                    psc[:],
                    mybir.ActivationFunctionType.Identity,
                    bias=0.0,
                )

            # ---------------- softmax over T (free axis) ----------------
            mx = smp.tile([BC, 1], F32)
            nc.vector.tensor_reduce(
                mx[:], scores_s[:], axis=mybir.AxisListType.X, op=mybir.AluOpType.max
            )
            nmx = smp.tile([BC, 1], F32)
            nc.vector.tensor_scalar_mul(nmx[:], mx[:], -1.0)
            attn = smp.tile([BC, T], F32)
            zsum = smp.tile([BC, 1], F32)
            nc.scalar.activation(
                attn[:],
                scores_s[:],
                mybir.ActivationFunctionType.Exp,
                bias=nmx[:],
                accum_out=zsum[:],
            )
            rz = smp.tile([BC, 1], F32)
            nc.vector.reciprocal(rz[:], zsum[:])
            attn_bf = smp.tile([BC, T], BF16)
            nc.vector.tensor_scalar_mul(attn_bf[:], attn[:], rz[:])

            # ---------------- pass B ----------------
            for b in range(BC):
                for p in range(NPAIR):
                    pa2 = pa2p.tile([128, CH], F32, tag="pa2")
                    for h in range(2):
                        cs = (2 * p + h) * CH
                        nc.tensor.matmul(
                            pa2[h * 64 : h * 64 + 64, :],
                            bcast_s[:, b * 64 : (b + 1) * 64],
                            attn_bf[:, cs : cs + CH],
                            start=True,
                            stop=True,
                        )
                    tav = pbp.tile([128, CH], F32, tag="tav")
                    nc.vector.tensor_mul(tav[:], pa2[:], vstore[:, stsl(b, p)])
                    # accumulate the fp32 eeg residual via SWDGE compute-DMA
                    nc.gpsimd.dma_start(
                        tav[:], xstore[:, stsl(b, p)], accum_op=mybir.AluOpType.add
                    )
                    nc.sync.dma_start(out_d[b, p], tav[:])

    nc.compile()
    return nc


def _get_nc():
    if "nc" not in _CACHE:
        _CACHE["nc"] = _build_nc()
    return _CACHE["nc"]


def _host_constants(Wq, bq, Wk, bk, Wv, bv):
    Wq64, Wk64, Wv64 = (np.asarray(a, np.float64) for a in (Wq, Wk, Wv))
    bq64, bk64 = np.asarray(bq, np.float64), np.asarray(bk, np.float64)
    G = SCALE * (Wq64.T @ Wk64)  # [d, e]
    w = SCALE * (Wq64.T @ bk64)  # [64]
    u = SCALE * (Wk64.T @ bq64)  # [64]

    BIG = np.zeros((128, 128), np.float64)
    BIG[0:64, 0:64] = G  # z[e,n] = sum_d G[d,e] x[d,n]
    BIG[64:128, 64:128] = np.asarray(Wv, np.float64).T  # v[o,n] = sum_e Wv[o,e] y[e,n]

    MASK1 = np.zeros((64, 64), np.float32)
    MASKW = np.zeros((64, 64), np.float32)
    for b in range(BC):
        MASK1[:, b * 8 + b] = 1.0
        MASKW[:, b * 8 + b] = w
    U = np.tile(u.reshape(64, 1), (2, 1)).astype(np.float32)
    BV2 = np.tile(np.asarray(bv, np.float32).reshape(64, 1), (2, 1))
    BCAST = np.zeros((BC, BC * 64), np.float32)
    for b in range(BC):
        BCAST[b, b * 64 : (b + 1) * 64] = 1.0
    return (
        BIG.astype(NPBF16),
        MASK1.astype(NPBF16),
        MASKW.astype(NPBF16),
        U,
        BV2,
        BCAST.astype(NPBF16),
    )


def _pack_inputs(eeg, fnirs):
    # [T, B, D] -> [core, chunk, b, feat, tok]
    def tr(x):
        x = np.asarray(x, np.float32).reshape(NCH, CH, N_CORES, BC, D)
        return x.transpose(2, 0, 3, 4, 1)

    XY = np.empty((N_CORES, NCH, BC, 128, CH), NPBF16)
    XY[:, :, :, 0:64, :] = tr(eeg)
    XY[:, :, :, 64:128, :] = tr(fnirs)
    # fp32 eeg residual in pass-B layout: [core, b, pair, half*64+d, tok]
    e = np.asarray(eeg, np.float32).reshape(NPAIR, 2, CH, N_CORES, BC, D)
    EEGR = np.ascontiguousarray(e.transpose(3, 4, 0, 1, 5, 2)).reshape(
        N_CORES, BC, NPAIR, 128, CH
    )
    return XY, EEGR


def _unpack_output(outs):
    # outs: list of [BC, NPAIR, 128, CH] per core -> [T, B, D]
    o = np.stack(outs)  # [core, b, pair, 128, tok]
    o = o.reshape(N_CORES, BC, NPAIR, 2, D, CH)  # [core, b, pair, half, d, tok]
    o = o.transpose(2, 3, 5, 0, 1, 4)  # [pair, half, tok, core, b, d]
    return np.ascontiguousarray(o.reshape(T, B, D))


def _run(eeg, fnirs, Wq, bq, Wk, bk, Wv, bv, **spmd_kwargs):
    from concourse.bass_utils import run_bass_kernel_spmd

    nc = _get_nc()
    BIG, MASK1, MASKW, U, BV2, BCAST = _host_constants(Wq, bq, Wk, bk, Wv, bv)
    XY, EEGR = _pack_inputs(eeg, fnirs)
    in_maps = [
        {
            "XY": np.ascontiguousarray(XY[c]),
            "EEGR": EEGR[c],
            "BIG": BIG,
            "MASK1": MASK1,
            "MASKW": MASKW,
            "UVEC": U,
            "BV2": BV2,
            "BCAST": BCAST,
        }
        for c in range(N_CORES)
    ]
    res = run_bass_kernel_spmd(nc, in_maps, list(range(N_CORES)), **spmd_kwargs)
    return _unpack_output([res.results[c]["OUT"] for c in range(N_CORES)]), res


def kernel(eeg, fnirs, Wq, bq, Wk, bk, Wv, bv):
    return _run(eeg, fnirs, Wq, bq, Wk, bk, Wv, bv)[0]


# revision 14
# speedup vs baseline: 1.7070x; 1.5818x over previous
"""CrossAttentionFusion Bass kernel for 8 TRN2 NeuronCores.

Reference computation (T=4096, B=64, D=64):
    q = eeg @ Wq.T + bq ; k = fnirs @ Wk.T + bk ; v = fnirs @ Wv.T + bv
    score = sum(q*k, -1) * D**-0.5        # [T, B, 1]
    attn = softmax(score, axis=0)         # over T
    out = eeg + attn * v

Strategy:
  - Data-parallel over batch: core c handles batches [8c, 8c+8).
  - Algebraic fold: score*SCALE = x^T G y + w.x + u.y (+const, dropped —
    softmax shift-invariant), with G = SCALE*Wq^T@Wk, w = SCALE*Wq^T@bk,
    u = SCALE*Wk^T@bq.  This removes the q/k projections entirely.
  - Everything on-device runs in [feature, token] (transposed) layout; the
    host packs eeg/fnirs into stacked bf16 [128, 2048] superchunk tiles
    (x on partitions 0:64, y on 64:128; 4 KB DMA descriptors).  One
    block-diagonal bf16 matmul [[G,0],[0,Wv^T]] (fp32 PSUM accumulate)
    per 512-token slice produces z (=G^T x) and v.
  - Per-batch masked reduce-matmuls (column-one-hot lhsT) accumulate
    scores for all 8 local batches into a single PSUM tile [8, 512].
  - Softmax over T with batch on partitions; per-chunk partial maxima are
    reduced during pass A, exp+sum and normalization run per 512-slice.
  - Pass B: PE one-hot-row matmul broadcasts attn rows across 64
    partitions, DVE multiplies by v (fp32), DVE/GpSimd (alternating) add
    the fp32 eeg residual (prefetched to SBUF during pass A), DMA out.
"""

import sys

sys.path.insert(0, "/opt/trn_rl_repo")

import ml_dtypes
import numpy as np

import concourse.bass as bass
import concourse.tile as tile
from concourse import bacc, mybir

T, B, D = 4096, 64, 64
N_CORES = 8
BC = B // N_CORES  # 8 batches per core
NCH = 8  # 512-token chunks along T
CH = T // NCH  # 512
NSC = 2  # superchunks (DMA granularity)
SCW = T // NSC // CH  # 4 chunks per superchunk
NPAIR = NCH // 2  # chunk pairs (128-partition packing in pass B)
SCALE = float(D) ** -0.5
F32 = mybir.dt.float32
BF16 = mybir.dt.bfloat16
NPBF16 = ml_dtypes.bfloat16

_CACHE = {}


def _build_nc():
    nc = bacc.Bacc(
        "TRN2", target_bir_lowering=False, debug=False, num_devices=N_CORES
    )

    xy_d = nc.dram_tensor(
        "XY", [NSC, BC, 128, SCW * CH], BF16, kind="ExternalInput"
    ).ap()
    eegr_d = nc.dram_tensor(
        "EEGR", [BC, 128, NPAIR * CH], F32, kind="ExternalInput"
    ).ap()
    big_d = nc.dram_tensor("BIG", [128, 128], BF16, kind="ExternalInput").ap()
    mask1_d = nc.dram_tensor("MASK1", [64, 64], BF16, kind="ExternalInput").ap()
    maskw_d = nc.dram_tensor("MASKW", [64, 64], BF16, kind="ExternalInput").ap()
    u_d = nc.dram_tensor("UVEC", [128, 1], F32, kind="ExternalInput").ap()
    bv_d = nc.dram_tensor("BV2", [128, 1], F32, kind="ExternalInput").ap()
    bcast_d = nc.dram_tensor(
        "BCAST", [BC, BC * 64], BF16, kind="ExternalInput"
    ).ap()
    out_d = nc.dram_tensor(
        "OUT", [BC, NPAIR, 128, CH], F32, kind="ExternalOutput"
    ).ap()

    with tile.TileContext(nc) as tc:
        with (
            tc.tile_pool(name="consts", bufs=1) as consts,
            tc.tile_pool(name="store", bufs=1) as store,
            tc.tile_pool(name="xy", bufs=3) as xyp,
            tc.tile_pool(name="m", bufs=6) as mp,
            tc.tile_pool(name="sm", bufs=1) as smp,
            tc.tile_pool(name="passb", bufs=4) as pbp,
            tc.tile_pool(name="pzv", bufs=3, space="PSUM") as pzvp,
            tc.tile_pool(name="psc", bufs=4, space="PSUM") as pscp,
        ):
            big_s = consts.tile([128, 128], BF16)
            nc.sync.dma_start(big_s[:], big_d[:])
            mask1_s = consts.tile([64, 64], BF16)
            nc.sync.dma_start(mask1_s[:], mask1_d[:])
            maskw_s = consts.tile([64, 64], BF16)
            nc.sync.dma_start(maskw_s[:], maskw_d[:])
            u_s = consts.tile([128, 1], F32)
            nc.sync.dma_start(u_s[:], u_d[:])
            bv_s = consts.tile([128, 1], F32)
            nc.sync.dma_start(bv_s[:], bv_d[:])
            bcast_s = consts.tile([BC, BC * 64], BF16)
            nc.sync.dma_start(bcast_s[:], bcast_d[:])

            # persistent stores: eeg (fp32 residual) and v, [128, 512] x 32
            xstore = store.tile([128, NPAIR * BC * CH], F32)  # 64 KB/part
            vstore = store.tile([128, NPAIR * BC * CH], F32)  # 64 KB/part
            scores_s = smp.tile([BC, T], F32)
            mxp = smp.tile([BC, NCH], F32)  # per-chunk max partials
            zsp = smp.tile([BC, NCH], F32)  # per-chunk expsum partials

            def stsl(bi, pi):
                return slice((bi * NPAIR + pi) * CH, (bi * NPAIR + pi + 1) * CH)

            # ---------------- pass A ----------------
            for sc in range(NSC):
                pscs = [
                    pscp.tile([BC, CH], F32, tag="psc", name=f"psc_{sc}_{i}")
                    for i in range(SCW)
                ]
                for b in range(BC):
                    xy = xyp.tile([128, SCW * CH], BF16, tag="xy")
                    nc.sync.dma_start(xy[:], xy_d[sc, b])
                    if sc == 0:
                        # prefetch fp32 eeg residual for pass B; emitted here
                        # so it trails the first compute-critical loads
                        nc.sync.dma_start(
                            xstore[:, b * NPAIR * CH : (b + 1) * NPAIR * CH],
                            eegr_d[b],
                        )
                    for cq in range(SCW):
                        c = sc * SCW + cq
                        csl = slice(cq * CH, (cq + 1) * CH)
                        pzv = pzvp.tile([128, CH], F32, tag="pzv")
                        nc.tensor.matmul(
                            pzv[:], big_s[:], xy[:, csl], start=True, stop=True
                        )
                        half = slice((c % 2) * 64, (c % 2) * 64 + 64)
                        # v = Wv@y + bv, keep for pass B
                        nc.scalar.activation(
                            vstore[half, stsl(b, c // 2)],
                            pzv[64:128, :],
                            mybir.ActivationFunctionType.Identity,
                            bias=bv_s[half, :],
                        )
                        # scores[b, :] += w.x  (independent of DVE -> first)
                        nc.tensor.matmul(
                            pscs[cq][:],
                            maskw_s[:, b * 8 : b * 8 + 8],
                            xy[0:64, csl],
                            start=(b == 0),
                            stop=False,
                        )
                        # m = (z + u) * y
                        m = mp.tile([64, CH], BF16, tag="m")
                        nc.vector.scalar_tensor_tensor(
                            m[:],
                            pzv[0:64, :],
                            u_s[64:128, :],
                            xy[64:128, csl],
                            op0=mybir.AluOpType.add,
                            op1=mybir.AluOpType.mult,
                        )
                        # scores[b, :] += sum_d m
                        nc.tensor.matmul(
                            pscs[cq][:],
                            mask1_s[:, b * 8 : b * 8 + 8],
                            m[:],
                            start=False,
                            stop=(b == BC - 1),
                        )
                for cq in range(SCW):
                    c = sc * SCW + cq
                    ssl = slice(c * CH, (c + 1) * CH)
                    nc.scalar.activation(
                        scores_s[:, ssl],
                        pscs[cq][:],
                        mybir.ActivationFunctionType.Identity,
                        bias=0.0,
                    )
                    nc.vector.tensor_reduce(
                        mxp[:, c : c + 1],
                        scores_s[:, ssl],
                        axis=mybir.AxisListType.X,
                        op=mybir.AluOpType.max,
                    )

            # ---------------- softmax over T (free axis) ----------------
            nmx = smp.tile([BC, 1], F32)
            mx = smp.tile([BC, 1], F32)
            nc.vector.tensor_reduce(
                mx[:], mxp[:], axis=mybir.AxisListType.X, op=mybir.AluOpType.max
            )
            nc.vector.tensor_scalar_mul(nmx[:], mx[:], -1.0)
            attn_e = smp.tile([BC, T], BF16)
            for c in range(NCH):
                ssl = slice(c * CH, (c + 1) * CH)
                nc.scalar.activation(
                    attn_e[:, ssl],
                    scores_s[:, ssl],
                    mybir.ActivationFunctionType.Exp,
                    bias=nmx[:],
                    accum_out=zsp[:, c : c + 1],
                )
            zsum = smp.tile([BC, 1], F32)
            nc.vector.tensor_reduce(
                zsum[:], zsp[:], axis=mybir.AxisListType.X, op=mybir.AluOpType.add
            )
            rz = smp.tile([BC, 1], F32)
            nc.vector.reciprocal(rz[:], zsum[:])
            attn_bf = smp.tile([BC, T], BF16)
            for c in range(NCH):
                ssl = slice(c * CH, (c + 1) * CH)
                nc.vector.tensor_scalar_mul(
                    attn_bf[:, ssl], attn_e[:, ssl], rz[:]
                )

            # ---------------- pass B ----------------
            for b in range(BC):
                for p in range(NPAIR):
                    pa2 = pzvp.tile([128, CH], F32, tag="pzv")
                    for h in range(2):
                        cs = (2 * p + h) * CH
                        nc.tensor.matmul(
                            pa2[h * 64 : h * 64 + 64, :],
                            bcast_s[:, b * 64 : (b + 1) * 64],
                            attn_bf[:, cs : cs + CH],
                            start=True,
                            stop=True,
                        )
                    tav = pbp.tile([128, CH], F32, tag="tav")
                    nc.vector.tensor_mul(tav[:], pa2[:], vstore[:, stsl(b, p)])
                    o2 = pbp.tile([128, CH], F32, tag="o2")
                    eng = nc.vector if (p % 2 == 0) else nc.gpsimd
                    eng.tensor_add(o2[:], tav[:], xstore[:, stsl(b, p)])
                    nc.sync.dma_start(out_d[b, p], o2[:])

    nc.compile()
    return nc


def _get_nc():
    if "nc" not in _CACHE:
        _CACHE["nc"] = _build_nc()
    return _CACHE["nc"]


def _host_constants(Wq, bq, Wk, bk, Wv, bv):
    Wq64, Wk64, Wv64 = (np.asarray(a, np.float64) for a in (Wq, Wk, Wv))
    bq64, bk64 = np.asarray(bq, np.float64), np.asarray(bk, np.float64)
    G = SCALE * (Wq64.T @ Wk64)  # [d, e]
    w = SCALE * (Wq64.T @ bk64)  # [64]
    u = SCALE * (Wk64.T @ bq64)  # [64]

    BIG = np.zeros((128, 128), np.float64)
    BIG[0:64, 0:64] = G  # z[e,n] = sum_d G[d,e] x[d,n]
    BIG[64:128, 64:128] = np.asarray(Wv, np.float64).T  # v[o,n] = sum_e Wv[o,e] y[e,n]

    MASK1 = np.zeros((64, 64), np.float32)
    MASKW = np.zeros((64, 64), np.float32)
    for b in range(BC):
        MASK1[:, b * 8 + b] = 1.0
        MASKW[:, b * 8 + b] = w
    U = np.tile(u.reshape(64, 1), (2, 1)).astype(np.float32)
    BV2 = np.tile(np.asarray(bv, np.float32).reshape(64, 1), (2, 1))
    BCAST = np.zeros((BC, BC * 64), np.float32)
    for b in range(BC):
        BCAST[b, b * 64 : (b + 1) * 64] = 1.0
    return (
        BIG.astype(NPBF16),
        MASK1.astype(NPBF16),
        MASKW.astype(NPBF16),
        U,
        BV2,
        BCAST.astype(NPBF16),
    )


def _pack_inputs(eeg, fnirs):
    # [T, B, D] -> XY[core, sc, b, feat, SCW*CH]; tok index = cq*CH + t
    def tr(x):
        x = np.asarray(x, np.float32).reshape(NSC, SCW, CH, N_CORES, BC, D)
        # -> [core, sc, b, d, cq, t]
        x = x.transpose(3, 0, 4, 5, 1, 2)
        return x.reshape(N_CORES, NSC, BC, D, SCW * CH)

    XY = np.empty((N_CORES, NSC, BC, 128, SCW * CH), NPBF16)
    XY[:, :, :, 0:64, :] = tr(eeg)
    XY[:, :, :, 64:128, :] = tr(fnirs)
    # fp32 eeg residual, pass-B layout: [core, b, half*64+d, p*CH+t]
    e = np.asarray(eeg, np.float32).reshape(NPAIR, 2, CH, N_CORES, BC, D)
    e = e.transpose(3, 4, 1, 5, 0, 2)  # [core, b, half, d, pair, tok]
    EEGR = np.ascontiguousarray(e).reshape(N_CORES, BC, 128, NPAIR * CH)
    return XY, EEGR


def _unpack_output(outs):
    # outs: list of [BC, NPAIR, 128, CH] per core -> [T, B, D]
    o = np.stack(outs)  # [core, b, pair, 128, tok]
    o = o.reshape(N_CORES, BC, NPAIR, 2, D, CH)  # [core, b, pair, half, d, tok]
    o = o.transpose(2, 3, 5, 0, 1, 4)  # [pair, half, tok, core, b, d]
    return np.ascontiguousarray(o.reshape(T, B, D))


def _run(eeg, fnirs, Wq, bq, Wk, bk, Wv, bv, **spmd_kwargs):
    from concourse.bass_utils import run_bass_kernel_spmd

    nc = _get_nc()
    BIG, MASK1, MASKW, U, BV2, BCAST = _host_constants(Wq, bq, Wk, bk, Wv, bv)
    XY, EEGR = _pack_inputs(eeg, fnirs)
    in_maps = [
        {
            "XY": np.ascontiguousarray(XY[c]),
            "EEGR": EEGR[c],
            "BIG": BIG,
            "MASK1": MASK1,
            "MASKW": MASKW,
            "UVEC": U,
            "BV2": BV2,
            "BCAST": BCAST,
        }
        for c in range(N_CORES)
    ]
    res = run_bass_kernel_spmd(nc, in_maps, list(range(N_CORES)), **spmd_kwargs)
    return _unpack_output([res.results[c]["OUT"] for c in range(N_CORES)]), res


def kernel(eeg, fnirs, Wq, bq, Wk, bk, Wv, bv):
    return _run(eeg, fnirs, Wq, bq, Wk, bk, Wv, bv)[0]
